# revision 31
# baseline (speedup 1.0000x reference)
"""Multi-head attention TRN2 kernel (B=4, S=2048, E=1024, H=16, D=64) on 8 cores.

Sharding: core c handles (batch b = c//2, query-half hq = c%2). Each core gets
the full batch-b sequence (rotated so its query half is rows 0-1023 -- softmax
over keys is order-invariant) and computes y rows for its 1024 queries. No
collectives; outputs concatenate.

v2 changes vs baseline:
  - Fused phase 0+A: per s-tile transpose + V-projection; V kept resident in
    SBUF as bf16 [128, ST, 16, 65] (ones col 64 preset) -- no vdram round-trip.
  - exp split across engines: ACT does cols 0:SPLIT (bf16 out), DVE does
    SPLIT:1024 via a one-pass Schraudolph exp (tensor_scalar mult+add ->
    int16, round-to-nearest, bitcast bf16).
  - qt/kt/pt/attnV in bf16 (scores + attnV matmuls bf16; QK-JIT/V-proj f32r).
  - aout bf16; W_out preloaded + cast to bf16 during phase B; phase C matmuls
    N=1024.
  - normalization muls offloaded to gpsimd.
"""
from contextlib import ExitStack

import numpy as np

import concourse.bass as bass
import concourse.tile as tile
from concourse import bacc, mybir
from concourse.bass_utils import run_bass_kernel_spmd
from concourse.masks import make_identity

F32R = mybir.dt.float32r
F32 = mybir.dt.float32
BF16 = mybir.dt.bfloat16
I16 = mybir.dt.int16
AF = mybir.ActivationFunctionType
ALU = mybir.AluOpType

B, S, E, H, D = 4, 2048, 1024, 16, 64
Q = 1024          # queries per core
ET = 8            # e-tiles (contraction over E)
ST = 16           # s-tiles of the sequence
KT = 16           # k-tiles in attention
NP = 8            # head-pairs
N_CORES = 8

SPLIT = 640       # exp columns done by ACT; rest by DVE Schraudolph

_LN2 = float(np.log(2.0))
A16 = 0.125 * (2.0**23) / _LN2 / 65536.0
B16 = float(127 * 2**23 - 366393) / 65536.0


def _bcast_dram(ap1d, n_part, n_free):
    """Broadcast a DRAM row across n_part partitions: [[0,n_part],[1,n_free]]."""
    return bass.AP(
        tensor=ap1d.tensor, offset=ap1d.offset, ap=[[0, n_part], [1, n_free]]
    )


def _emit(tc, nc, x, wqkv, bqkv, wout, bout, y, rscr, rscr2):
    with ExitStack() as ctx:
        xt_pool = ctx.enter_context(tc.tile_pool(name="xt", bufs=1))
        vsb_pool = ctx.enter_context(tc.tile_pool(name="vsb", bufs=1))
        const = ctx.enter_context(tc.tile_pool(name="const", bufs=1))

        xt = xt_pool.tile([128, ET, S], BF16)
        v_sb = vsb_pool.tile([128, ST, H, D + 1], BF16)

        ident = const.tile([128, 128], BF16)
        make_identity(nc, ident)
        bqk_t = const.tile([128, 24], F32)
        nc.sync.dma_start(out=bqk_t, in_=bqkv.rearrange("(j p) -> p j", p=128).bitcast(F32))
        bout_t = const.tile([128, E], F32)
        nc.scalar.dma_start(out=bout_t, in_=_bcast_dram(bout[0:1], 128, E))
        # softmax-denominator ones column of V
        nc.gpsimd.memset(v_sb[:, :, :, D : D + 1], 1.0)

        # ---- fused phase 0+A: per s-tile transpose then V = x @ Wv + bv ----
        ph0 = ExitStack()
        wvp = ph0.enter_context(tc.tile_pool(name="wv", bufs=1))
        bvp = ph0.enter_context(tc.tile_pool(name="bv", bufs=1))
        wv = wvp.tile([128, ET, E], BF16)

        def load_wv(ch):
            nc.gpsimd.dma_start(
                out=wv[:, ch * 2 : (ch + 1) * 2, :],
                in_=wqkv[:, 2 * E : 3 * E]
                .rearrange("(t p) n -> p t n", p=128)[:, ch * 2 : (ch + 1) * 2, :],
            )

        bv_t = bvp.tile([128, H, D], F32R)
        nc.scalar.dma_start(out=bv_t, in_=_bcast_dram(bqkv[2 * E : 2 * E + 1], 128, E))
        with (
            tc.tile_pool(name="xload", bufs=6) as xload,
            tc.tile_pool(name="tps", bufs=2, space="PSUM") as tps,
            tc.tile_pool(name="vps", bufs=3, space="PSUM") as vps,
        ):
            def v_proj(st):
                vp_ps = vps.tile([128, H, D], F32)
                for half in range(2):
                    for et in range(ET):
                        nc.tensor.matmul(
                            vp_ps[:, half * 8 : (half + 1) * 8, :],
                            xt[:, et, st * 128 : (st + 1) * 128],
                            wv[:, et, half * 512 : (half + 1) * 512],
                            start=(et == 0),
                            stop=(et == ET - 1),
                        )
                nc.vector.tensor_add(v_sb[:, st, :, 0:D], vp_ps, bv_t)

            for st in range(ST):
                xs = xload.tile([128, E], BF16)
                if st == 0:
                    # split the cold first load so transposes start sooner
                    for quarter in range(4):
                        nc.gpsimd.dma_start(
                            out=xs[:, quarter * 256 : (quarter + 1) * 256],
                            in_=x[0:128, quarter * 256 : (quarter + 1) * 256],
                        )
                else:
                    nc.gpsimd.dma_start(out=xs, in_=x[st * 128 : (st + 1) * 128, :])
                if st in (1, 2):
                    load_wv(2 * (st - 1))
                    load_wv(2 * (st - 1) + 1)
                for g in range(2):
                    ps = tps.tile([128, 4, 128], BF16)
                    for i in range(4):
                        et = g * 4 + i
                        nc.tensor.transpose(
                            ps[:, i, :], xs[:, et * 128 : (et + 1) * 128], ident
                        )
                    nc.vector.tensor_copy(
                        xt[:, g * 4 : (g + 1) * 4, st * 128 : (st + 1) * 128], ps
                    )
                if st >= 2:
                    v_proj(st - 2)
            v_proj(ST - 2)
            v_proj(ST - 1)
        ph0.close()

        # ---- W_out preload (fp32 chunks -> bf16), overlapped with phase B ----
        wo_pool = ctx.enter_context(tc.tile_pool(name="wo", bufs=1))
        wo_bf = wo_pool.tile([128, ET, E], BF16)

        aout_pool = ctx.enter_context(tc.tile_pool(name="aout", bufs=1))
        aout = aout_pool.tile([128, NP, Q], BF16)

        wo_thunks = []
        for qtr in range(4):
            def load_qtr(qtr=qtr):
                nc.gpsimd.dma_start(
                    out=wo_bf[:, :, qtr * 256 : (qtr + 1) * 256],
                    in_=wout[:, qtr * 256 : (qtr + 1) * 256].rearrange(
                        "(t p) n -> p t n", p=128
                    ),
                )
            wo_thunks.append(load_qtr)

        with (
            tc.tile_pool(name="wqk", bufs=2) as wqkp,
            tc.tile_pool(name="qt", bufs=2) as qtp,
            tc.tile_pool(name="kt", bufs=2) as ktp,
            tc.tile_pool(name="pt", bufs=4) as ptp,
            tc.tile_pool(name="ev", bufs=1) as evp,
            tc.tile_pool(name="qkps", bufs=1, space="PSUM") as qkps,
            tc.tile_pool(name="scps", bufs=2, space="PSUM") as scps,
            tc.tile_pool(name="accps", bufs=2, space="PSUM") as accps,
        ):

            def build_pair(p):
                """Allocate pair-p input tiles; return (tiles, emission thunks)."""
                wq = wqkp.tile([128, ET, 128], BF16, tag="wq")
                wk = wqkp.tile([128, ET, 128], BF16, tag="wk")
                qt_t = qtp.tile([128, Q], BF16)
                kt_t = ktp.tile([128, S], BF16)
                th = []
                th.append(lambda: nc.gpsimd.dma_start(
                    out=wq,
                    in_=wqkv[:, p * 128 : (p + 1) * 128].rearrange(
                        "(t p2) m -> p2 t m", p2=128),
                ))
                th.append(lambda: nc.gpsimd.dma_start(
                    out=wk,
                    in_=wqkv[:, E + p * 128 : E + (p + 1) * 128].rearrange(
                        "(t p2) m -> p2 t m", p2=128),
                ))

                def qk_group(dst, w, bias_col, xoff):
                    g = []
                    ps_box = []

                    def alloc():
                        qk_ps = qkps.tile([128, 1024], F32, tag="qk")
                        ps_box.append(qk_ps)
                    g.append(alloc)
                    for half in range(2):
                        for et in range(ET):
                            g.append(lambda half=half, et=et: nc.tensor.matmul(
                                ps_box[0][:, half * 512 : (half + 1) * 512],
                                w[:, et, :],
                                xt[:, et, xoff + half * 512 : xoff + (half + 1) * 512],
                                start=(et == 0),
                                stop=(et == ET - 1),
                            ))
                    g.append(lambda: nc.vector.tensor_scalar_add(dst, ps_box[0], bias_col))
                    return g

                th += qk_group(qt_t, wq, bqk_t[:, p : p + 1], 0)
                th += qk_group(kt_t[:, 0:1024], wk, bqk_t[:, 8 + p : 9 + p], 0)
                th += qk_group(kt_t[:, 1024:2048], wk, bqk_t[:, 8 + p : 9 + p], 1024)
                return {"qt": qt_t, "kt": kt_t}, th

            def phase_c_tile(qt_i):
                """Thunks computing y rows for one 128-query tile via qkps."""
                th = []
                box = []

                def alloc():
                    box.append(qkps.tile([128, E], F32, tag="qk", name="yc"))
                th.append(alloc)
                for half in range(2):
                    for p8 in range(8):
                        th.append(lambda half=half, p8=p8: nc.tensor.matmul(
                            box[0][:, half * 512 : (half + 1) * 512],
                            aout[:, p8, qt_i * 128 : (qt_i + 1) * 128],
                            wo_bf[:, p8, half * 512 : (half + 1) * 512],
                            start=(p8 == 0), stop=(p8 == 7),
                        ))

                def evict():
                    yb = evp.tile([128, E], F32, tag="yb", name="yb")
                    nc.vector.tensor_add(yb, box[0], bout_t)
                    nc.sync.dma_start(
                        out=y[qt_i * 128 : (qt_i + 1) * 128, :], in_=yb
                    )
                th.append(evict)
                return th

            cur, th0 = build_pair(0)
            for t in th0:
                t()

            for p in range(NP):
                if p + 1 < NP:
                    nxt, pending = build_pair(p + 1)
                else:
                    nxt = None
                    pending = phase_c_tile(0)
                pending = list(pending)
                if p == 0:
                    pending += wo_thunks
                qt_t, kt_t = cur["qt"], cur["kt"]
                for qh in range(2):
                    qsl = slice(qh * 512, (qh + 1) * 512)
                    acc0 = accps.tile([128, 512], F32, tag="acc")
                    acc1 = accps.tile([128, 512], F32, tag="acc")
                    pts = [None] * KT

                    def attn_v(kk):
                        nc.tensor.matmul(
                            acc0[0:65, :], v_sb[:, kk, 2 * p, :], pts[kk][:, 0:512],
                            start=(kk == 0), stop=(kk == KT - 1),
                        )
                        nc.tensor.matmul(
                            acc1[0:65, :], v_sb[:, kk, 2 * p + 1, :],
                            pts[kk][:, 512:1024],
                            start=(kk == 0), stop=(kk == KT - 1),
                        )

                    for k2 in range(0, KT, 2):
                        scs = []
                        for k in (k2, k2 + 1):
                            sc = scps.tile([128, 1024], F32, tag="sc")
                            scs.append(sc)
                            nc.tensor.matmul(
                                sc[:, 0:512],
                                kt_t[0:64, k * 128 : (k + 1) * 128],
                                qt_t[0:64, qsl],
                                start=True, stop=True,
                            )
                            nc.tensor.matmul(
                                sc[:, 512:1024],
                                kt_t[64:128, k * 128 : (k + 1) * 128],
                                qt_t[64:128, qsl],
                                start=True, stop=True,
                            )
                        if k2 >= 2:
                            attn_v(k2 - 2)
                            attn_v(k2 - 1)
                        for k in (k2, k2 + 1):
                            sc = scs[k - k2]
                            pt_t = ptp.tile([128, 1024], BF16)
                            pts[k] = pt_t
                            nc.scalar.activation(
                                out=pt_t[:, 0:SPLIT], in_=sc[:, 0:SPLIT],
                                func=AF.Exp, scale=0.125,
                            )
                            nc.vector.tensor_scalar(
                                out=pt_t.bitcast(I16)[:, SPLIT:1024],
                                in0=sc[:, SPLIT:1024],
                                scalar1=A16, scalar2=B16,
                                op0=ALU.mult, op1=ALU.add,
                            )
                        npop = 4
                        if p == NP - 1:
                            # phase-C tile-0 thunks: only once pair-7 qh0's
                            # eviction (which they read) is safely complete
                            npop = 5 if (qh == 1 and k2 >= 8) else 0
                        for _ in range(npop):
                            if pending:
                                pending.pop(0)()
                    attn_v(KT - 2)
                    attn_v(KT - 1)
                    # eviction: fast psum release, then off-path normalization
                    ridx = p * 2 + qh
                    au0 = evp.tile([128, 512], F32, tag="au0")
                    nc.vector.tensor_copy(au0[0:65, :], acc0[0:65, :])
                    au1 = evp.tile([128, 512], F32, tag="au1")
                    nc.vector.tensor_copy(au1[0:65, :], acc1[0:65, :])
                    nc.sync.dma_start(out=rscr[ridx : ridx + 1, 0:512], in_=au0[64:65, :])
                    nc.sync.dma_start(out=rscr[ridx : ridx + 1, 512:1024], in_=au1[64:65, :])
                    rw = evp.tile([64, 16], F32, tag="rw")
                    nc.sync.dma_start(
                        out=rw, in_=rscr[ridx : ridx + 1, :].rearrange("o (p f) -> (o p) f", p=64)
                    )
                    rwr = evp.tile([64, 16], F32, tag="rwr")
                    nc.vector.reciprocal(rwr, rw)
                    nc.sync.dma_start(
                        out=rscr2[ridx : ridx + 1, :].rearrange("o (p f) -> (o p) f", p=64),
                        in_=rwr,
                    )
                    sc0 = evp.tile([64, 512], F32, tag="sc0")
                    nc.sync.dma_start(out=sc0, in_=_bcast_dram(rscr2[ridx, 0:1], 64, 512))
                    sc1 = evp.tile([64, 512], F32, tag="sc1")
                    nc.sync.dma_start(out=sc1, in_=_bcast_dram(rscr2[ridx, 512:513], 64, 512))
                    nc.gpsimd.tensor_tensor(
                        out=aout[0:64, p, qsl], in0=au0[0:64, :], in1=sc0, op=ALU.mult
                    )
                    tmp1 = evp.tile([64, 512], BF16, tag="tmp1")
                    nc.gpsimd.tensor_tensor(
                        out=tmp1, in0=au1[0:64, :], in1=sc1, op=ALU.mult
                    )
                    nc.sync.dma_start(out=aout[64:128, p, qsl], in_=tmp1)
                for t in pending:
                    t()
                cur = nxt

        # ---- phase C: y = attn_out @ W_out + b_out ----
        with (
            tc.tile_pool(name="yps", bufs=2, space="PSUM") as yps,
            tc.tile_pool(name="yev", bufs=3) as yev,
        ):
            for qt_i in range(1, 8):
                ps = yps.tile([128, E], F32)
                for half in range(2):
                    for p8 in range(8):
                        nc.tensor.matmul(
                            ps[:, half * 512 : (half + 1) * 512],
                            aout[:, p8, qt_i * 128 : (qt_i + 1) * 128],
                            wo_bf[:, p8, half * 512 : (half + 1) * 512],
                            start=(p8 == 0),
                            stop=(p8 == 7),
                        )
                yb = yev.tile([128, E], F32)
                nc.vector.tensor_add(yb, ps, bout_t)
                nc.sync.dma_start(
                    out=y[qt_i * 128 : (qt_i + 1) * 128, :],
                    in_=yb,
                )


def build_nc():
    nc = bacc.Bacc("TRN2", target_bir_lowering=False, debug=False)
    x = nc.dram_tensor("x", [S, E], F32R, kind="ExternalInput").ap()
    wqkv = nc.dram_tensor("wqkv", [E, 3 * E], F32R, kind="ExternalInput").ap()
    bqkv = nc.dram_tensor("bqkv", [3 * E], F32R, kind="ExternalInput").ap()
    wout = nc.dram_tensor("wout", [E, E], F32, kind="ExternalInput").ap()
    bout = nc.dram_tensor("bout", [E], F32, kind="ExternalInput").ap()
    y = nc.dram_tensor("y", [Q, E], F32, kind="ExternalOutput").ap()
    rscr = nc.dram_tensor("rscr", [16, 1024], F32).ap()
    rscr2 = nc.dram_tensor("rscr2", [16, 1024], F32).ap()
    with tile.TileContext(nc) as tc:
        _emit(tc, nc, x, wqkv, bqkv, wout, bout, y, rscr, rscr2)
    nc.compile()
    return nc


_NC = None


def _get_nc():
    global _NC
    if _NC is None:
        _NC = build_nc()
    return _NC


def make_in_maps(x, W_qkv, b_qkv, W_out, b_out):
    x = np.ascontiguousarray(np.asarray(x, dtype=np.float32))
    W_qkv = np.ascontiguousarray(np.asarray(W_qkv, dtype=np.float32))
    b_qkv = np.ascontiguousarray(np.asarray(b_qkv, dtype=np.float32))
    W_out = np.ascontiguousarray(np.asarray(W_out, dtype=np.float32))
    b_out = np.ascontiguousarray(np.asarray(b_out, dtype=np.float32))
    in_maps = []
    for c in range(N_CORES):
        b, hq = c // 2, c % 2
        xb = x[b]
        if hq:
            xb = np.ascontiguousarray(np.concatenate([xb[1024:], xb[:1024]], axis=0))
        in_maps.append(
            {"x": xb, "wqkv": W_qkv, "bqkv": b_qkv, "wout": W_out, "bout": b_out}
        )
    return in_maps


def assemble(results):
    out = np.empty((B, S, E), dtype=np.float32)
    for c in range(N_CORES):
        b, hq = c // 2, c % 2
        out[b, hq * 1024 : (hq + 1) * 1024, :] = results[c]["y"]
    return out


def kernel(x, W_qkv, b_qkv, W_out, b_out):
    nc = _get_nc()
    in_maps = make_in_maps(x, W_qkv, b_qkv, W_out, b_out)
    res = run_bass_kernel_spmd(nc, in_maps, list(range(N_CORES)))
    return assemble(res.results)
